# revision 2
# baseline (speedup 1.0000x reference)
"""Trainium2 Bass kernel for nn_BasicDecoder (cross-attention + MLP decoder block).

v2. Sharding: 8 cores; core c owns batch b = c//2 and head-group g = c%2
(4 heads). Reference's raw view reshape [B,H,Q,DH]->[B,Q,H*DH] makes output
row-block j depend only on head j, so each core computes a disjoint
[4096, 512] slice of the final output with no cross-core communication.

Differences vs v1:
 - q-LayerNorm folded into the Q projection: raw q streams into the PE;
   per-head correction Qh = r*P - (mu*r)*colsum(Wq) applied on DVE from
   stat tiles pre-replicated in the stats pre-pass (keeps the per-subtile
   critical path off the PE).
 - Q/K biases + kv-LN bias folds dropped via softmax shift-invariance
   (exact; per-token constants cancel). Nonzero-bq fallback via exp-bias.
 - Softmax denominator via matmul on pairwise pre-added exp tiles
   (4 denominator matmuls per (i,h) instead of 8).
 - ao/xn staging DMAs ride the gpsimd (SWDGE) queue, weights on sync.
 - outT is fp16 (host upcasts).
"""
import numpy as np
import ml_dtypes

import concourse.bass as bass
import concourse.tile as tile
from concourse import bacc, mybir
from concourse import bass_utils

F32 = mybir.dt.float32
FP16 = mybir.dt.float16
AF = mybir.ActivationFunctionType
ALU = mybir.AluOpType

B, Q, KV, D, H = 4, 8192, 1024, 1024, 8
DH = D // H            # 128
OUT_C = 512
HID = 4096
EPS = 1e-5
N_CORES = 8
HPC = H // 2           # heads per core = 4
ROWS = Q // 2          # output rows per core = 4096
SUB = 512
NSUB = Q // SUB        # 16 qtok subtiles
NSTRIP = 2             # strips of 4096 qtok

_CACHE = {}


def _query_perm():
    """perm[P] = original qtok index at permuted position P."""
    s = np.arange(NSTRIP)[:, None, None]
    u = np.arange(8)[None, :, None]
    rho = np.arange(SUB)[None, None, :]
    return (4096 * s + 8 * rho + u).reshape(-1)


def build(nrep=1, qbias=False, stages=("kv", "pre", "att", "wo", "p2")):
    nc = bacc.Bacc("TRN2", target_bir_lowering=False, debug=False,
                   enable_asserts=False)

    def din(name, shape, dt=FP16):
        return nc.dram_tensor(name, shape, dt, kind="ExternalInput").ap()

    qT = din("qT", [D, Q])
    zT = din("zT", [D, KV])
    wq = din("wq", [D, 512]); wk = din("wk", [D, 512]); wv = din("wv", [D, 512])
    nwq = din("nwq", [128, HPC], F32)
    wo = din("wo", [D, D])
    w1 = din("w1", [D, HID], FP16)
    w2 = din("w2", [HID, D], FP16)
    wf = din("wf", [D, OUT_C])
    bvb = din("bvb", [128, 512], F32)
    bo = din("bo", [128, 8], F32)
    b1 = din("b1", [128, 32], F32)
    b2 = din("b2", [128, 8], F32)
    bfp = din("bfp", [128, 4], F32)
    if qbias:
        bqh = din("bqh", [128, HPC], F32)

    outT = nc.dram_tensor("outT", [OUT_C, ROWS], FP16, kind="ExternalOutput").ap()

    # [p, c, t] views of [D, N] dram tensors (D = 8 chunks x 128 partitions)
    qTv = qT.rearrange("(c p) t -> p c t", p=128)
    zTv = zT.rearrange("(c p) t -> p c t", p=128)
    wqv = wq.rearrange("(c p) n -> c p n", p=128)
    wkv = wk.rearrange("(c p) n -> c p n", p=128)
    wvv = wv.rearrange("(c p) n -> c p n", p=128)
    wov = wo.rearrange("(c p) n -> c p n", p=128)
    w1v = w1.rearrange("(c p) n -> p c n", p=128)
    w2v = w2.rearrange("(c p) n -> c p n", p=128)
    wfv = wf.rearrange("(c p) n -> c p n", p=128)

    with tile.TileContext(nc) as tc:
        with tc.tile_pool(name="outer", bufs=1) as outer, \
             tc.tile_pool(name="dstg", bufs=1, space="DRAM") as dstg:
            # ---- constants & biases ----
            ones_f = outer.tile([128, 128], F32)
            nc.gpsimd.memset(ones_f[:], 1.0)
            ones_col = outer.tile([128, 1], FP16)
            nc.vector.tensor_copy(ones_col[:], ones_f[:, 0:1])
            ones_row = outer.tile([1, 128], FP16)
            nc.vector.tensor_copy(ones_row[:], ones_f[0:1, :])
            ones128 = outer.tile([128, 128], FP16)
            nc.vector.tensor_copy(ones128[:], ones_f[:])
            nwq_t = outer.tile([128, HPC], F32); nc.sync.dma_start(nwq_t[:], nwq)
            bvb_t = outer.tile([128, 512], F32); nc.sync.dma_start(bvb_t[:], bvb)
            bo_t = outer.tile([128, 8], F32); nc.sync.dma_start(bo_t[:], bo)
            b1_t = outer.tile([128, 32], F32); nc.sync.dma_start(b1_t[:], b1)
            b2_t = outer.tile([128, 8], F32); nc.sync.dma_start(b2_t[:], b2)
            bf_t = outer.tile([128, 4], F32); nc.sync.dma_start(bf_t[:], bfp)
            if qbias:
                bq_t = outer.tile([128, HPC], F32)
                nc.sync.dma_start(bq_t[:], bqh)
                bq16 = outer.tile([128, HPC], FP16)
                nc.vector.tensor_copy(bq16[:], bq_t[:])

            ao_stg = dstg.tile([8, 128, 8, SUB], FP16)
            xn_stg = dstg.tile([8, 128, 8, SUB], FP16)

            def ln_core(ps1, sbp, rhs_chunks, n_feat, width, sq_maker):
                """Shared LN stats: returns (mu [1,w] fp16, r [1,w] fp16)."""
                s_ps = ps1.tile([1, width], F32, tag="stat_s")
                q_ps = ps1.tile([1, width], F32, tag="stat_q")
                nch = len(rhs_chunks)
                for c in range(nch):
                    nc.tensor.matmul(s_ps[:], ones_col[:], rhs_chunks[c],
                                     start=(c == 0), stop=(c == nch - 1))
                for c in range(nch):
                    nc.tensor.matmul(q_ps[:], ones_col[:], sq_maker(c),
                                     start=(c == 0), stop=(c == nch - 1))
                mu = sbp.tile([1, width], FP16, tag="lmu")
                nc.vector.tensor_scalar_mul(mu[:], s_ps[:], 1.0 / n_feat)
                tmp = sbp.tile([1, width], F32, tag="ltmp")
                nc.vector.tensor_tensor(tmp[:], mu[:], s_ps[:], op=ALU.mult)
                v = sbp.tile([1, width], F32, tag="lvar")
                nc.vector.tensor_tensor(v[:], q_ps[:], tmp[:], op=ALU.subtract)
                ve = sbp.tile([1, width], F32, tag="lve")
                nc.vector.tensor_scalar(ve[:], v[:], 1.0 / n_feat, EPS,
                                        op0=ALU.mult, op1=ALU.add)
                vr = sbp.tile([1, width], F32, tag="lvr")
                nc.vector.reciprocal(vr[:], ve[:])
                r = sbp.tile([1, width], FP16, tag="lr")
                nc.scalar.activation(r[:], vr[:], AF.Sqrt)
                return mu, r

            def replicate(ps2, sbp, row_ap, width, tag, ps_tag="repl"):
                rp = ps2.tile([128, width], F32, tag=ps_tag)
                nc.tensor.matmul(rp[:], ones_row[:], row_ap, start=True, stop=True)
                out = sbp.tile([128, 1, width], FP16, tag=tag)
                nc.vector.tensor_copy(out[:, 0], rp[:])
                return out

            for _rep in range(nrep):
              with tc.tile_pool(name="pers", bufs=1) as pers:
                    O_str = {}
                    for s in range(NSTRIP):
                        for h in range(HPC):
                            ostr_tile = pers.tile([128, 4096], FP16,
                                                  tag=f"o{s}{h}")
                            O_str[(s, h)] = ostr_tile

                    with tc.tile_pool(name="kvw", bufs=1) as kvw, \
                         tc.tile_pool(name="rrm2", bufs=1) as rrm2:
                        rr_all = rrm2.tile([128, NSUB, SUB], FP16, tag="rrall")
                        m2_all = rrm2.tile([128, NSUB, SUB], FP16, tag="m2all")
                        wq_sb = kvw.tile([128, 8, 512], FP16, tag="wq")
                        K_sb = kvw.tile([128, HPC, KV], FP16, tag="K")
                        V_sb = kvw.tile([128, 8, 512], FP16, tag="V")
                        if qbias:
                            c1_sb = kvw.tile([128, HPC, 8], F32, tag="c1")
                        for c in range(8):
                            nc.sync.dma_start(wq_sb[:, c], wqv[c])

                        # ================= KV stage =================
                        if "kv" in stages:
                          with tc.tile_pool(name="kvp1", bufs=1) as kvp1, \
                             tc.tile_pool(name="kvp2", bufs=2) as kvp2, \
                             tc.tile_pool(name="kvps1", bufs=1, space="PSUM") as kvps1, \
                             tc.tile_pool(name="kvps2", bufs=2, space="PSUM") as kvps2:
                            wk_sb = kvp1.tile([128, 8, 512], FP16, tag="wk")
                            wv_sb = kvp1.tile([128, 8, 512], FP16, tag="wv")
                            for c in range(8):
                                nc.sync.dma_start(wk_sb[:, c], wkv[c])
                                nc.sync.dma_start(wv_sb[:, c], wvv[c])
                            zt = kvp1.tile([128, 8, KV], FP16, tag="zt")
                            nc.sync.dma_start(zt[:], zTv)
                            for hf in range(2):
                                sl = slice(hf * 512, hf * 512 + 512)

                                def sqm(c, sl=sl):
                                    t = kvp2.tile([128, 512], FP16, tag="zsq")
                                    nc.scalar.activation(t[:], zt[:, c, sl],
                                                         AF.Square)
                                    return t[:]
                                mu, r = ln_core(kvps1, kvp2,
                                                [zt[:, c, sl] for c in range(8)],
                                                D, 512, sqm)
                                mur = replicate(kvps2, kvp2, mu[:], 512, "murep")
                                rr = replicate(kvps2, kvp2, r[:], 512, "rrep")
                                t1 = kvp2.tile([128, 8, 512], FP16, tag="kt1")
                                nc.vector.tensor_tensor(
                                    t1[:], zt[:, :, sl],
                                    mur[:].to_broadcast((128, 8, 512)),
                                    op=ALU.subtract)
                                nc.vector.tensor_tensor(
                                    zt[:, :, sl], t1[:],
                                    rr[:].to_broadcast((128, 8, 512)),
                                    op=ALU.mult)
                            for h in range(HPC):
                                for hf in range(2):
                                    sl = slice(hf * 512, hf * 512 + 512)
                                    kps = kvps2.tile([128, 512], F32, tag="kwork")
                                    for c in range(8):
                                        nc.tensor.matmul(
                                            kps[:], wk_sb[:, c, 128 * h:128 * h + 128],
                                            zt[:, c, sl], start=(c == 0), stop=(c == 7))
                                    nc.vector.tensor_copy(K_sb[:, h, sl], kps[:])
                            if qbias:
                                for h in range(HPC):
                                    for c in range(8):
                                        cps = kvps2.tile([128, 1], F32, tag="repl")
                                        nc.tensor.matmul(
                                            cps[:], K_sb[:, h, 128 * c:128 * c + 128],
                                            bq16[:, h:h + 1], start=True, stop=True)
                                        nc.vector.tensor_copy(c1_sb[:, h, c:c + 1],
                                                              cps[:])
                            for kc in range(8):
                                vps = kvps2.tile([128, 512], F32, tag="vwork")
                                for c in range(8):
                                    nc.tensor.matmul(
                                        vps[:], zt[:, c, 128 * kc:128 * kc + 128],
                                        wv_sb[:, c], start=(c == 0), stop=(c == 7))
                                nc.vector.tensor_tensor(
                                    V_sb[:, kc], vps[:], bvb_t[:], op=ALU.add)

                        # ============ q-LN stats pre-pass ============
                        if "pre" in stages:
                          with tc.tile_pool(name="qsp", bufs=2) as qsp, \
                             tc.tile_pool(name="qsps", bufs=3, space="PSUM") as qsps, \
                             tc.tile_pool(name="qspr", bufs=2, space="PSUM") as qspr:
                            for i in range(NSUB):
                                qt = qsp.tile([128, 8, SUB], FP16, tag="qt")
                                nc.sync.dma_start(qt[:],
                                                  qTv[:, :, SUB * i:SUB * (i + 1)])

                                sqtiles = {}
                                def sqm(c):
                                    j, jj = divmod(c, 2)
                                    if jj == 0:
                                        t = qsp.tile([128, 2, SUB], FP16, tag="qsq")
                                        if j % 2 == 0:
                                            nc.scalar.activation(
                                                t[:], qt[:, 2 * j:2 * j + 2],
                                                AF.Square)
                                        else:
                                            nc.vector.tensor_tensor(
                                                t[:], qt[:, 2 * j:2 * j + 2],
                                                qt[:, 2 * j:2 * j + 2],
                                                op=ALU.mult)
                                        sqtiles[j] = t
                                    return sqtiles[j][:, jj]
                                mu, r = ln_core(qsps, qsp,
                                                [qt[:, c] for c in range(8)],
                                                D, SUB, sqm)
                                m2 = qsp.tile([1, SUB], FP16, tag="m2row")
                                nc.vector.tensor_tensor(m2[:], mu[:], r[:],
                                                        op=ALU.mult)
                                rrep = qspr.tile([128, SUB], F32, tag="repl")
                                nc.tensor.matmul(rrep[:], ones_row[:], r[:],
                                                 start=True, stop=True)
                                nc.vector.tensor_copy(rr_all[:, i], rrep[:])
                                mrep = qspr.tile([128, SUB], F32, tag="repl")
                                nc.tensor.matmul(mrep[:], ones_row[:], m2[:],
                                                 start=True, stop=True)
                                nc.vector.tensor_copy(m2_all[:, i], mrep[:])

                        # ================= attention =================
                        if "att" in stages:
                          with tc.tile_pool(name="attp", bufs=3) as attp, \
                             tc.tile_pool(name="pcp", bufs=6) as pcp, \
                             tc.tile_pool(name="psP", bufs=2, space="PSUM") as psP, \
                             tc.tile_pool(name="psSC", bufs=2, space="PSUM") as psSC, \
                             tc.tile_pool(name="psO", bufs=2, space="PSUM") as psO:
                            for i in range(NSUB):
                                s, isub = divmod(i, 8)
                                qt = attp.tile([128, 8, SUB], FP16, tag="qt")
                                nc.sync.dma_start(qt[:],
                                                  qTv[:, :, SUB * i:SUB * (i + 1)])
                                for h in range(HPC):
                                    P = psP.tile([128, SUB], F32, tag="P")
                                    for c in range(8):
                                        nc.tensor.matmul(
                                            P[:], wq_sb[:, c, 128 * h:128 * h + 128],
                                            qt[:, c], start=(c == 0), stop=(c == 7))
                                    qa = attp.tile([128, SUB], FP16, tag="qa")
                                    nc.vector.tensor_tensor(
                                        qa[:], P[:], rr_all[:, i], op=ALU.mult)
                                    Qh = attp.tile([128, SUB], FP16, tag="Qh")
                                    nc.vector.scalar_tensor_tensor(
                                        Qh[:], m2_all[:, i], nwq_t[:, h:h + 1],
                                        qa[:], op0=ALU.mult, op1=ALU.add)
                                    ops = psO.tile([128, SUB], F32, tag="ops")
                                    dps = psO.tile([128, SUB], F32, tag="dps")
                                    pcs = []
                                    for c in range(8):
                                        sc = psSC.tile([128, SUB], F32, tag="sc")
                                        nc.tensor.matmul(
                                            sc[:], K_sb[:, h, 128 * c:128 * c + 128],
                                            Qh[:], start=True, stop=True)
                                        pc = pcp.tile([128, SUB], FP16, tag="pc")
                                        if qbias:
                                            nc.scalar.activation(
                                                pc[:], sc[:], AF.Exp,
                                                bias=c1_sb[:, h, c:c + 1])
                                        else:
                                            nc.scalar.activation(pc[:], sc[:],
                                                                 AF.Exp)
                                        pcs.append(pc)
                                        nc.tensor.matmul(
                                            ops[:], V_sb[:, c, 128 * h:128 * h + 128],
                                            pc[:], start=(c == 0), stop=(c == 7))
                                        if c % 2 == 1:
                                            p2t = pcp.tile([128, SUB], FP16,
                                                           tag="pc2")
                                            nc.vector.tensor_tensor(
                                                p2t[:], pcs[c - 1][:], pc[:],
                                                op=ALU.add)
                                            nc.tensor.matmul(
                                                dps[:], ones128[:], p2t[:],
                                                start=(c == 1), stop=(c == 7))
                                    rec = attp.tile([128, SUB], F32, tag="rec")
                                    nc.vector.reciprocal(rec[:], dps[:])
                                    nc.vector.tensor_tensor(
                                        O_str[(s, h)][:, SUB * isub:SUB * (isub + 1)],
                                        ops[:], rec[:], op=ALU.mult)

                    # ================= Wo + attn LN =================
                    if "wo" in stages:
                      with tc.tile_pool(name="wop1", bufs=1) as wop1, \
                         tc.tile_pool(name="wop2", bufs=2) as wop2, \
                         tc.tile_pool(name="wop3", bufs=1) as wop3, \
                         tc.tile_pool(name="wops1", bufs=1, space="PSUM") as wops1, \
                         tc.tile_pool(name="wops2", bufs=2, space="PSUM") as wops2:
                        wo_sb = wop1.tile([128, 8, D], FP16, tag="wo")
                        for c in range(8):
                            nc.sync.dma_start(wo_sb[:, c], wov[c])
                        for s in range(NSTRIP):
                            for h in range(HPC):
                                t = s * HPC + h
                                AO = wop1.tile([128, 8, SUB], FP16, tag="AO")
                                for oc in range(8):
                                    aps = wops2.tile([128, SUB], F32, tag="aops")
                                    for u in range(8):
                                        nc.tensor.matmul(
                                            aps[:],
                                            wo_sb[:, u, 128 * oc:128 * oc + 128],
                                            O_str[(s, h)][:, SUB * u:SUB * (u + 1)],
                                            start=(u == 0), stop=(u == 7))
                                    nc.vector.tensor_scalar_add(
                                        AO[:, oc], aps[:], bo_t[:, oc:oc + 1])
                                nc.gpsimd.dma_start(ao_stg[t], AO[:])

                                sqt = {}
                                def sqm(c):
                                    j, jj = divmod(c, 2)
                                    if jj == 0:
                                        tq = wop2.tile([128, 2, SUB], FP16,
                                                       tag="aosq")
                                        nc.scalar.activation(
                                            tq[:], AO[:, 2 * j:2 * j + 2],
                                            AF.Square)
                                        sqt[j] = tq
                                    return sqt[j][:, jj]
                                mu, r = ln_core(wops1, wop3,
                                                [AO[:, c] for c in range(8)],
                                                D, SUB, sqm)
                                mur = replicate(wops2, wop3, mu[:], SUB, "murep")
                                rr = replicate(wops2, wop3, r[:], SUB, "rrep")
                                t1 = wop2.tile([128, 8, SUB], FP16, tag="xn")
                                xn = wop2.tile([128, 8, SUB], FP16, tag="xn")
                                nc.vector.tensor_tensor(
                                    t1[:], AO[:],
                                    mur[:].to_broadcast((128, 8, SUB)),
                                    op=ALU.subtract)
                                nc.vector.tensor_tensor(
                                    xn[:], t1[:],
                                    rr[:].to_broadcast((128, 8, SUB)),
                                    op=ALU.mult)
                                nc.gpsimd.dma_start(xn_stg[t], xn[:])

              # ================= MLP + final projection =================
              if "p2" in stages:
                with tc.tile_pool(name="w1p", bufs=1) as w1p, \
                   tc.tile_pool(name="w2p", bufs=1) as w2p, \
                   tc.tile_pool(name="p2h", bufs=1) as p2h, \
                   tc.tile_pool(name="p2b", bufs=2) as p2b, \
                   tc.tile_pool(name="p2ps", bufs=3, space="PSUM") as p2ps, \
                   tc.tile_pool(name="p2psx", bufs=1, space="PSUM") as p2psx:
                  w1_sb = w1p.tile([128, 8, HID], FP16, tag="w1")
                  nc.sync.dma_start(w1_sb[:], w1v)
                  w2_sb = w2p.tile([128, 32, D], FP16, tag="w2")
                  for c in range(32):
                      nc.sync.dma_start(w2_sb[:, c], w2v[c])
                  wf_sb = w2p.tile([128, 8, OUT_C], FP16, tag="wf")
                  for c in range(8):
                      nc.sync.dma_start(wf_sb[:, c], wfv[c])
                  for t in range(8):
                      s2, h2 = divmod(t, HPC)
                      rowoff = 1024 * h2 + 512 * s2
                      xn_t = p2h.tile([128, 8, SUB], FP16, tag="xnin")
                      nc.gpsimd.dma_start(xn_t[:], xn_stg[t])
                      ao_t = p2b.tile([128, 8, SUB], FP16, tag="aot")
                      nc.gpsimd.dma_start(ao_t[:], ao_stg[t])
                      h_sb = p2h.tile([128, 32, SUB], FP16, tag="h")
                      for G in range(32):
                          hps = p2ps.tile([128, SUB], F32, tag="hps")
                          for c in range(8):
                              nc.tensor.matmul(
                                  hps[:],
                                  w1_sb[:, c, 128 * G:128 * G + 128],
                                  xn_t[:, c], start=(c == 0), stop=(c == 7))
                          nc.scalar.activation(h_sb[:, G], hps[:], AF.Gelu,
                                               bias=b1_t[:, G:G + 1])
                      X = p2h.tile([128, 8, SUB], FP16, tag="X")
                      for half in range(2):
                          xps = p2psx.tile([128, 4, SUB], F32, tag="xps")
                          for G in range(32):
                              for oc4 in range(4):
                                  oc = 4 * half + oc4
                                  nc.tensor.matmul(
                                      xps[:, oc4],
                                      w2_sb[:, G, 128 * oc:128 * oc + 128],
                                      h_sb[:, G], start=(G == 0), stop=(G == 31))
                          for oc4 in range(4):
                              oc = 4 * half + oc4
                              nc.vector.scalar_tensor_tensor(
                                  X[:, oc], xps[:, oc4], b2_t[:, oc:oc + 1],
                                  ao_t[:, oc],
                                  op0=ALU.add, op1=ALU.add)
                      for of in range(4):
                          ofps = p2ps.tile([128, SUB], F32, tag="hps")
                          for c in range(8):
                              nc.tensor.matmul(
                                  ofps[:], wf_sb[:, c, 128 * of:128 * of + 128],
                                  X[:, c], start=(c == 0), stop=(c == 7))
                          outt = p2b.tile([128, SUB], FP16, tag="outt")
                          nc.vector.tensor_scalar_add(outt[:], ofps[:],
                                                      bf_t[:, of:of + 1])
                          nc.sync.dma_start(
                              outT[128 * of:128 * (of + 1),
                                   rowoff:rowoff + SUB], outt[:])
    nc.compile()
    return nc


def _prep_host(inputs):
    """Fold LN gains + attention scale into weights; build per-core maps."""
    f64 = np.float64
    gq, bq_ln = inputs["ln_q_g"].astype(f64), inputs["ln_q_b"].astype(f64)
    gkv = inputs["ln_kv_g"].astype(f64)
    bkv_ln = inputs["ln_kv_b"].astype(f64)
    ga, ba_ln = inputs["ln_a_g"].astype(f64), inputs["ln_a_b"].astype(f64)
    Wq, Wk, Wv = (np.asarray(inputs[k], f64) for k in ("Wq", "Wk", "Wv"))
    Wo, W1, W2, Wf = (np.asarray(inputs[k], f64) for k in ("Wo", "W1", "W2", "Wf"))
    bq_, bv_ = (np.asarray(inputs[k], f64) for k in ("bq", "bv"))
    bo_, b1_, b2_, bf_ = (np.asarray(inputs[k], f64)
                          for k in ("bo", "b1", "b2", "bf"))

    sc = 1.0 / np.sqrt(DH)
    Wq_e = (gq[:, None] * Wq) * sc
    bq_e = (bq_ln @ Wq + bq_) * sc
    Wk_e = gkv[:, None] * Wk
    Wv_e = gkv[:, None] * Wv
    bv_e = bkv_ln @ Wv + bv_
    W1_e = ga[:, None] * W1
    b1_e = ba_ln @ W1 + b1_

    qbias = bool(np.abs(bq_e).max() > 1e-12)

    perm = _query_perm()
    f32 = np.float32
    query = np.asarray(inputs["query"], f32)
    z = np.asarray(inputs["z"], f32)
    maps = []
    shared = {
        "wo": np.ascontiguousarray(Wo.astype(np.float16)),
        "w1": np.ascontiguousarray(W1_e.astype(np.float16)),
        "w2": np.ascontiguousarray(W2.astype(np.float16)),
        "wf": np.ascontiguousarray(Wf.astype(np.float16)),
        "bo": np.ascontiguousarray(bo_.reshape(8, 128).T.astype(f32)),
        "b1": np.ascontiguousarray(b1_e.reshape(32, 128).T.astype(f32)),
        "b2": np.ascontiguousarray(b2_.reshape(8, 128).T.astype(f32)),
        "bfp": np.ascontiguousarray(bf_.reshape(4, 128).T.astype(f32)),
    }
    for core in range(N_CORES):
        b, g = divmod(core, 2)
        hs = slice(512 * g, 512 * (g + 1))
        m = dict(shared)
        nwq_v = -(Wq_e[:, hs].sum(axis=0))
        m.update({
            "qT": np.ascontiguousarray(query[b][perm].T.astype(np.float16)),
            "zT": np.ascontiguousarray(z[b].T.astype(np.float16)),
            "wq": np.ascontiguousarray(Wq_e[:, hs].astype(np.float16)),
            "wk": np.ascontiguousarray(Wk_e[:, hs].astype(np.float16)),
            "wv": np.ascontiguousarray(Wv_e[:, hs].astype(np.float16)),
            "nwq": np.ascontiguousarray(nwq_v.reshape(HPC, 128).T.astype(f32)),
            "bvb": np.broadcast_to(bv_e[hs].astype(f32), (128, 512)).copy(),
        })
        if qbias:
            m["bqh"] = np.ascontiguousarray(
                bq_e[hs].reshape(HPC, 128).T.astype(f32))
        maps.append(m)
    return maps, qbias


def kernel(**inputs):
    assert bool(np.all(inputs["query_mask"])), \
        "kernel specialization assumes all-ones query_mask"
    maps, qbias = _prep_host(inputs)
    key = ("nc", qbias)
    if key not in _CACHE:
        _CACHE[key] = build(qbias=qbias)
    nc = _CACHE[key]
    res = bass_utils.run_bass_kernel_spmd(nc, maps, core_ids=list(range(N_CORES)))
    out = np.empty((B, Q, OUT_C), dtype=np.float32)
    for core in range(N_CORES):
        b, g = divmod(core, 2)
        out[b, ROWS * g:ROWS * (g + 1), :] = res.results[core]["outT"].T
    return out


# revision 3
# speedup vs baseline: 1.3169x; 1.3169x over previous
"""Trainium2 Bass kernel for nn_BasicDecoder (cross-attention + MLP decoder block).

v2. Sharding: 8 cores; core c owns batch b = c//2 and head-group g = c%2
(4 heads). Reference's raw view reshape [B,H,Q,DH]->[B,Q,H*DH] makes output
row-block j depend only on head j, so each core computes a disjoint
[4096, 512] slice of the final output with no cross-core communication.

Differences vs v1:
 - q-LayerNorm folded into the Q projection: raw q streams into the PE;
   per-head correction Qh = r*P - (mu*r)*colsum(Wq) applied on DVE from
   stat tiles pre-replicated in the stats pre-pass (keeps the per-subtile
   critical path off the PE).
 - Q/K biases + kv-LN bias folds dropped via softmax shift-invariance
   (exact; per-token constants cancel). Nonzero-bq fallback via exp-bias.
 - Softmax denominator via matmul on pairwise pre-added exp tiles
   (4 denominator matmuls per (i,h) instead of 8).
 - ao/xn staging DMAs ride the gpsimd (SWDGE) queue, weights on sync.
 - outT is fp16 (host upcasts).
"""
import numpy as np
import ml_dtypes

import concourse.bass as bass
import concourse.tile as tile
from concourse import bacc, mybir
from concourse import bass_utils

F32 = mybir.dt.float32
FP16 = mybir.dt.float16
AF = mybir.ActivationFunctionType
ALU = mybir.AluOpType

B, Q, KV, D, H = 4, 8192, 1024, 1024, 8
DH = D // H            # 128
OUT_C = 512
HID = 4096
EPS = 1e-5
N_CORES = 8
HPC = H // 2           # heads per core = 4
ROWS = Q // 2          # output rows per core = 4096
SUB = 512
NSUB = Q // SUB        # 16 qtok subtiles
NSTRIP = 2             # strips of 4096 qtok

_CACHE = {}


def _query_perm():
    """perm[P] = original qtok index at permuted position P."""
    s = np.arange(NSTRIP)[:, None, None]
    u = np.arange(8)[None, :, None]
    rho = np.arange(SUB)[None, None, :]
    return (4096 * s + 8 * rho + u).reshape(-1)


def build(nrep=1, qbias=False, stages=("kv", "pre", "att", "wo", "p2")):
    nc = bacc.Bacc("TRN2", target_bir_lowering=False, debug=False,
                   enable_asserts=False)

    def din(name, shape, dt=FP16):
        return nc.dram_tensor(name, shape, dt, kind="ExternalInput").ap()

    qT = din("qT", [D, Q])
    zT = din("zT", [D, KV])
    wq = din("wq", [D, 512]); wk = din("wk", [D, 512]); wv = din("wv", [D, 512])
    nwq = din("nwq", [128, HPC], F32)
    wo = din("wo", [D, D])
    w1 = din("w1", [D, HID], FP16)
    w2 = din("w2", [HID, D], FP16)
    wf = din("wf", [D, OUT_C])
    bvb = din("bvb", [128, 512], F32)
    bo = din("bo", [128, 8], F32)
    b1 = din("b1", [128, 32], F32)
    b2 = din("b2", [128, 8], F32)
    bfp = din("bfp", [128, 4], F32)
    if qbias:
        bqh = din("bqh", [128, HPC], F32)

    outT = nc.dram_tensor("outT", [OUT_C, ROWS], FP16, kind="ExternalOutput").ap()

    # [p, c, t] views of [D, N] dram tensors (D = 8 chunks x 128 partitions)
    qTv = qT.rearrange("(c p) t -> p c t", p=128)
    zTv = zT.rearrange("(c p) t -> p c t", p=128)
    wqv = wq.rearrange("(c p) n -> c p n", p=128)
    wkv = wk.rearrange("(c p) n -> c p n", p=128)
    wvv = wv.rearrange("(c p) n -> c p n", p=128)
    wov = wo.rearrange("(c p) n -> c p n", p=128)
    w1v = w1.rearrange("(c p) n -> p c n", p=128)
    w2v = w2.rearrange("(c p) n -> c p n", p=128)
    wfv = wf.rearrange("(c p) n -> c p n", p=128)

    with tile.TileContext(nc) as tc:
        with tc.tile_pool(name="outer", bufs=1) as outer, \
             tc.tile_pool(name="dstg", bufs=1, space="DRAM") as dstg:
            # ---- constants & biases ----
            ones_f = outer.tile([128, 128], F32)
            nc.gpsimd.memset(ones_f[:], 1.0)
            ones_col = outer.tile([128, 1], FP16)
            nc.vector.tensor_copy(ones_col[:], ones_f[:, 0:1])
            ones_row = outer.tile([1, 128], FP16)
            nc.vector.tensor_copy(ones_row[:], ones_f[0:1, :])
            ones128 = outer.tile([128, 128], FP16)
            nc.vector.tensor_copy(ones128[:], ones_f[:])
            nwq_t = outer.tile([128, HPC], F32); nc.sync.dma_start(nwq_t[:], nwq)
            bvb_t = outer.tile([128, 512], F32); nc.sync.dma_start(bvb_t[:], bvb)
            bo_t = outer.tile([128, 8], F32); nc.sync.dma_start(bo_t[:], bo)
            b1_t = outer.tile([128, 32], F32); nc.sync.dma_start(b1_t[:], b1)
            b2_t = outer.tile([128, 8], F32); nc.sync.dma_start(b2_t[:], b2)
            bf_t = outer.tile([128, 4], F32); nc.sync.dma_start(bf_t[:], bfp)
            if qbias:
                bq_t = outer.tile([128, HPC], F32)
                nc.sync.dma_start(bq_t[:], bqh)
                bq16 = outer.tile([128, HPC], FP16)
                nc.vector.tensor_copy(bq16[:], bq_t[:])

            ao_stg = dstg.tile([8, 128, 8, SUB], FP16)
            xn_stg = dstg.tile([8, 128, 8, SUB], FP16)

            def ln_core(ps1, sbp, rhs_chunks, n_feat, width, sq_maker):
                """Shared LN stats: returns (mu [1,w] fp16, r [1,w] fp16)."""
                s_ps = ps1.tile([1, width], F32, tag="stat_s")
                q_ps = ps1.tile([1, width], F32, tag="stat_q")
                nch = len(rhs_chunks)
                for c in range(nch):
                    nc.tensor.matmul(s_ps[:], ones_col[:], rhs_chunks[c],
                                     start=(c == 0), stop=(c == nch - 1))
                for c in range(nch):
                    nc.tensor.matmul(q_ps[:], ones_col[:], sq_maker(c),
                                     start=(c == 0), stop=(c == nch - 1))
                mu = sbp.tile([1, width], FP16, tag="lmu")
                nc.vector.tensor_scalar_mul(mu[:], s_ps[:], 1.0 / n_feat)
                tmp = sbp.tile([1, width], F32, tag="ltmp")
                nc.vector.tensor_tensor(tmp[:], mu[:], s_ps[:], op=ALU.mult)
                v = sbp.tile([1, width], F32, tag="lvar")
                nc.vector.tensor_tensor(v[:], q_ps[:], tmp[:], op=ALU.subtract)
                ve = sbp.tile([1, width], F32, tag="lve")
                nc.vector.tensor_scalar(ve[:], v[:], 1.0 / n_feat, EPS,
                                        op0=ALU.mult, op1=ALU.add)
                vr = sbp.tile([1, width], F32, tag="lvr")
                nc.vector.reciprocal(vr[:], ve[:])
                r = sbp.tile([1, width], FP16, tag="lr")
                nc.scalar.activation(r[:], vr[:], AF.Sqrt)
                return mu, r

            def replicate(ps2, sbp, row_ap, width, tag, ps_tag="repl"):
                rp = ps2.tile([128, width], F32, tag=ps_tag)
                nc.tensor.matmul(rp[:], ones_row[:], row_ap, start=True, stop=True)
                out = sbp.tile([128, 1, width], FP16, tag=tag)
                nc.vector.tensor_copy(out[:, 0], rp[:])
                return out

            for _rep in range(nrep):
              with tc.tile_pool(name="pers", bufs=1) as pers:
                    O_str = {}
                    for s in range(NSTRIP):
                        for h in range(HPC):
                            ostr_tile = pers.tile([128, 4096], FP16,
                                                  tag=f"o{s}{h}")
                            O_str[(s, h)] = ostr_tile

                    with tc.tile_pool(name="kvw", bufs=1) as kvw, \
                         tc.tile_pool(name="rrm2", bufs=1) as rrm2:
                        rr_all = rrm2.tile([128, NSUB, SUB], FP16, tag="rrall")
                        m2_all = rrm2.tile([128, NSUB, SUB], FP16, tag="m2all")
                        wq_sb = kvw.tile([128, 8, 512], FP16, tag="wq")
                        K_sb = kvw.tile([128, HPC, KV], FP16, tag="K")
                        V_sb = kvw.tile([128, 8, 512], FP16, tag="V")
                        if qbias:
                            c1_sb = kvw.tile([128, HPC, 8], F32, tag="c1")
                        for c in range(8):
                            nc.sync.dma_start(wq_sb[:, c], wqv[c])

                        # ================= KV stage =================
                        if "kv" in stages:
                          with tc.tile_pool(name="kvp1", bufs=1) as kvp1, \
                             tc.tile_pool(name="kvp2", bufs=2) as kvp2, \
                             tc.tile_pool(name="kvps1", bufs=1, space="PSUM") as kvps1, \
                             tc.tile_pool(name="kvps2", bufs=2, space="PSUM") as kvps2:
                            wk_sb = kvp1.tile([128, 8, 512], FP16, tag="wk")
                            wv_sb = kvp1.tile([128, 8, 512], FP16, tag="wv")
                            for c in range(8):
                                nc.sync.dma_start(wk_sb[:, c], wkv[c])
                                nc.sync.dma_start(wv_sb[:, c], wvv[c])
                            zt = kvp1.tile([128, 8, KV], FP16, tag="zt")
                            nc.sync.dma_start(zt[:], zTv)
                            for hf in range(2):
                                sl = slice(hf * 512, hf * 512 + 512)

                                def sqm(c, sl=sl):
                                    t = kvp2.tile([128, 512], FP16, tag="zsq")
                                    nc.scalar.activation(t[:], zt[:, c, sl],
                                                         AF.Square)
                                    return t[:]
                                mu, r = ln_core(kvps1, kvp2,
                                                [zt[:, c, sl] for c in range(8)],
                                                D, 512, sqm)
                                mur = replicate(kvps2, kvp2, mu[:], 512, "murep")
                                rr = replicate(kvps2, kvp2, r[:], 512, "rrep")
                                t1 = kvp2.tile([128, 8, 512], FP16, tag="kt1")
                                nc.vector.tensor_tensor(
                                    t1[:], zt[:, :, sl],
                                    mur[:].to_broadcast((128, 8, 512)),
                                    op=ALU.subtract)
                                nc.vector.tensor_tensor(
                                    zt[:, :, sl], t1[:],
                                    rr[:].to_broadcast((128, 8, 512)),
                                    op=ALU.mult)
                            for h in range(HPC):
                                for hf in range(2):
                                    sl = slice(hf * 512, hf * 512 + 512)
                                    kps = kvps2.tile([128, 512], F32, tag="kwork")
                                    for c in range(8):
                                        nc.tensor.matmul(
                                            kps[:], wk_sb[:, c, 128 * h:128 * h + 128],
                                            zt[:, c, sl], start=(c == 0), stop=(c == 7))
                                    nc.vector.tensor_copy(K_sb[:, h, sl], kps[:])
                            if qbias:
                                for h in range(HPC):
                                    for c in range(8):
                                        cps = kvps2.tile([128, 1], F32, tag="repl")
                                        nc.tensor.matmul(
                                            cps[:], K_sb[:, h, 128 * c:128 * c + 128],
                                            bq16[:, h:h + 1], start=True, stop=True)
                                        nc.vector.tensor_copy(c1_sb[:, h, c:c + 1],
                                                              cps[:])
                            for kc in range(8):
                                vps = kvps2.tile([128, 512], F32, tag="vwork")
                                for c in range(8):
                                    nc.tensor.matmul(
                                        vps[:], zt[:, c, 128 * kc:128 * kc + 128],
                                        wv_sb[:, c], start=(c == 0), stop=(c == 7))
                                nc.vector.tensor_tensor(
                                    V_sb[:, kc], vps[:], bvb_t[:], op=ALU.add)

                        # ============ q-LN stats pre-pass ============
                        if "pre" in stages:
                          with tc.tile_pool(name="qsp", bufs=2) as qsp, \
                             tc.tile_pool(name="qsps", bufs=3, space="PSUM") as qsps, \
                             tc.tile_pool(name="qspr", bufs=2, space="PSUM") as qspr:
                            for i2 in range(NSUB // 2):
                              qt2 = qsp.tile([128, 8, 2 * SUB], FP16, tag="qt")
                              nc.sync.dma_start(
                                  qt2[:], qTv[:, :, 2 * SUB * i2:2 * SUB * (i2 + 1)])
                              for ii in range(2):
                                i = 2 * i2 + ii
                                qt = qt2[:, :, SUB * ii:SUB * (ii + 1)]

                                sqtiles = {}
                                def sqm(c):
                                    j, jj = divmod(c, 2)
                                    if jj == 0:
                                        t = qsp.tile([128, 2, SUB], FP16, tag="qsq")
                                        if j % 2 == 0:
                                            nc.scalar.activation(
                                                t[:], qt[:, 2 * j:2 * j + 2],
                                                AF.Square)
                                        else:
                                            nc.vector.tensor_tensor(
                                                t[:], qt[:, 2 * j:2 * j + 2],
                                                qt[:, 2 * j:2 * j + 2],
                                                op=ALU.mult)
                                        sqtiles[j] = t
                                    return sqtiles[j][:, jj]
                                mu, r = ln_core(qsps, qsp,
                                                [qt[:, c] for c in range(8)],
                                                D, SUB, sqm)
                                m2 = qsp.tile([1, SUB], FP16, tag="m2row")
                                nc.vector.tensor_tensor(m2[:], mu[:], r[:],
                                                        op=ALU.mult)
                                rrep = qspr.tile([128, SUB], F32, tag="repl")
                                nc.tensor.matmul(rrep[:], ones_row[:], r[:],
                                                 start=True, stop=True)
                                nc.vector.tensor_copy(rr_all[:, i], rrep[:])
                                mrep = qspr.tile([128, SUB], F32, tag="repl")
                                nc.tensor.matmul(mrep[:], ones_row[:], m2[:],
                                                 start=True, stop=True)
                                nc.vector.tensor_copy(m2_all[:, i], mrep[:])

                        # ================= attention =================
                        if "att" in stages:
                          with tc.tile_pool(name="attp", bufs=3) as attp, \
                             tc.tile_pool(name="pcp", bufs=6) as pcp, \
                             tc.tile_pool(name="psP", bufs=2, space="PSUM") as psP, \
                             tc.tile_pool(name="psSC", bufs=2, space="PSUM") as psSC, \
                             tc.tile_pool(name="psO", bufs=2, space="PSUM") as psO:
                            for i in range(NSUB):
                                s, isub = divmod(i, 8)
                                qt = attp.tile([128, 8, SUB], FP16, tag="qt")
                                nc.sync.dma_start(qt[:],
                                                  qTv[:, :, SUB * i:SUB * (i + 1)])
                                for h in range(HPC):
                                    P = psP.tile([128, SUB], F32, tag="P")
                                    for c in range(8):
                                        nc.tensor.matmul(
                                            P[:], wq_sb[:, c, 128 * h:128 * h + 128],
                                            qt[:, c], start=(c == 0), stop=(c == 7))
                                    qa = attp.tile([128, SUB], FP16, tag="qa")
                                    nc.vector.tensor_tensor(
                                        qa[:], P[:], rr_all[:, i], op=ALU.mult)
                                    Qh = attp.tile([128, SUB], FP16, tag="Qh")
                                    nc.vector.scalar_tensor_tensor(
                                        Qh[:], m2_all[:, i], nwq_t[:, h:h + 1],
                                        qa[:], op0=ALU.mult, op1=ALU.add)
                                    ops = psO.tile([128, SUB], F32, tag="ops")
                                    dps = psO.tile([128, SUB], F32, tag="dps")
                                    pcs = []
                                    p2s = []
                                    for c in range(8):
                                        sc = psSC.tile([128, SUB], F32, tag="sc")
                                        nc.tensor.matmul(
                                            sc[:], K_sb[:, h, 128 * c:128 * c + 128],
                                            Qh[:], start=True, stop=True)
                                        pc = pcp.tile([128, SUB], FP16, tag="pc")
                                        if qbias:
                                            nc.scalar.activation(
                                                pc[:], sc[:], AF.Exp,
                                                bias=c1_sb[:, h, c:c + 1])
                                        else:
                                            nc.scalar.activation(pc[:], sc[:],
                                                                 AF.Exp)
                                        pcs.append(pc)
                                        nc.tensor.matmul(
                                            ops[:], V_sb[:, c, 128 * h:128 * h + 128],
                                            pc[:], start=(c == 0), stop=(c == 7))
                                        if c % 2 == 1:
                                            p2t = pcp.tile([128, SUB], FP16,
                                                           tag="pc2")
                                            nc.vector.tensor_tensor(
                                                p2t[:], pcs[c - 1][:], pc[:],
                                                op=ALU.add)
                                            p2s.append(p2t)
                                        if c % 4 == 3:
                                            p4t = pcp.tile([128, SUB], FP16,
                                                           tag="pc4")
                                            nc.vector.tensor_tensor(
                                                p4t[:], p2s[-2][:], p2s[-1][:],
                                                op=ALU.add)
                                            nc.tensor.matmul(
                                                dps[:], ones128[:], p4t[:],
                                                start=(c == 3), stop=(c == 7))
                                    rec = attp.tile([128, SUB], F32, tag="rec")
                                    nc.vector.reciprocal(rec[:], dps[:])
                                    nc.vector.tensor_tensor(
                                        O_str[(s, h)][:, SUB * isub:SUB * (isub + 1)],
                                        ops[:], rec[:], op=ALU.mult)

                    # ================= Wo + attn LN =================
                    if "wo" in stages:
                      with tc.tile_pool(name="wop1", bufs=1) as wop1, \
                         tc.tile_pool(name="wop2", bufs=2) as wop2, \
                         tc.tile_pool(name="wop3", bufs=1) as wop3, \
                         tc.tile_pool(name="wops1", bufs=1, space="PSUM") as wops1, \
                         tc.tile_pool(name="wops2", bufs=2, space="PSUM") as wops2:
                        wo_sb = wop1.tile([128, 8, D], FP16, tag="wo")
                        for c in range(8):
                            nc.sync.dma_start(wo_sb[:, c], wov[c])
                        for s in range(NSTRIP):
                            for h in range(HPC):
                                t = s * HPC + h
                                AO = wop1.tile([128, 8, SUB], FP16, tag="AO")
                                for oc in range(8):
                                    aps = wops2.tile([128, SUB], F32, tag="aops")
                                    for u in range(8):
                                        nc.tensor.matmul(
                                            aps[:],
                                            wo_sb[:, u, 128 * oc:128 * oc + 128],
                                            O_str[(s, h)][:, SUB * u:SUB * (u + 1)],
                                            start=(u == 0), stop=(u == 7))
                                    nc.vector.tensor_scalar_add(
                                        AO[:, oc], aps[:], bo_t[:, oc:oc + 1])
                                nc.gpsimd.dma_start(ao_stg[t], AO[:])

                                sqt = {}
                                def sqm(c):
                                    j, jj = divmod(c, 2)
                                    if jj == 0:
                                        tq = wop2.tile([128, 2, SUB], FP16,
                                                       tag="aosq")
                                        nc.scalar.activation(
                                            tq[:], AO[:, 2 * j:2 * j + 2],
                                            AF.Square)
                                        sqt[j] = tq
                                    return sqt[j][:, jj]
                                mu, r = ln_core(wops1, wop3,
                                                [AO[:, c] for c in range(8)],
                                                D, SUB, sqm)
                                mur = replicate(wops2, wop3, mu[:], SUB, "murep")
                                rr = replicate(wops2, wop3, r[:], SUB, "rrep")
                                t1 = wop2.tile([128, 8, SUB], FP16, tag="xn")
                                xn = wop2.tile([128, 8, SUB], FP16, tag="xn")
                                nc.vector.tensor_tensor(
                                    t1[:], AO[:],
                                    mur[:].to_broadcast((128, 8, SUB)),
                                    op=ALU.subtract)
                                nc.vector.tensor_tensor(
                                    xn[:], t1[:],
                                    rr[:].to_broadcast((128, 8, SUB)),
                                    op=ALU.mult)
                                nc.gpsimd.dma_start(xn_stg[t], xn[:])

              # ================= MLP + final projection =================
              if "p2" in stages:
                with tc.tile_pool(name="w1p", bufs=1) as w1p, \
                   tc.tile_pool(name="w2p", bufs=1) as w2p, \
                   tc.tile_pool(name="p2h", bufs=1) as p2h, \
                   tc.tile_pool(name="p2b", bufs=2) as p2b, \
                   tc.tile_pool(name="p2ps", bufs=3, space="PSUM") as p2ps, \
                   tc.tile_pool(name="p2psx", bufs=1, space="PSUM") as p2psx:
                  w1_sb = w1p.tile([128, 8, HID], FP16, tag="w1")
                  nc.sync.dma_start(w1_sb[:], w1v)
                  w2_sb = w2p.tile([128, 32, D], FP16, tag="w2")
                  for c in range(32):
                      nc.sync.dma_start(w2_sb[:, c], w2v[c])
                  wf_sb = w2p.tile([128, 8, OUT_C], FP16, tag="wf")
                  for c in range(8):
                      nc.sync.dma_start(wf_sb[:, c], wfv[c])
                  for t in range(8):
                      s2, h2 = divmod(t, HPC)
                      rowoff = 1024 * h2 + 512 * s2
                      xn_t = p2h.tile([128, 8, SUB], FP16, tag="xnin")
                      nc.gpsimd.dma_start(xn_t[:], xn_stg[t])
                      ao_t = p2b.tile([128, 8, SUB], FP16, tag="aot")
                      nc.gpsimd.dma_start(ao_t[:], ao_stg[t])
                      h_sb = p2h.tile([128, 32, SUB], FP16, tag="h")
                      for G in range(32):
                          hps = p2ps.tile([128, SUB], F32, tag="hps")
                          for c in range(8):
                              nc.tensor.matmul(
                                  hps[:],
                                  w1_sb[:, c, 128 * G:128 * G + 128],
                                  xn_t[:, c], start=(c == 0), stop=(c == 7))
                          nc.scalar.activation(h_sb[:, G], hps[:], AF.Gelu,
                                               bias=b1_t[:, G:G + 1])
                      X = p2h.tile([128, 8, SUB], FP16, tag="X")
                      for half in range(2):
                          xps = p2psx.tile([128, 4, SUB], F32, tag="xps")
                          for G in range(32):
                              for oc4 in range(4):
                                  oc = 4 * half + oc4
                                  nc.tensor.matmul(
                                      xps[:, oc4],
                                      w2_sb[:, G, 128 * oc:128 * oc + 128],
                                      h_sb[:, G], start=(G == 0), stop=(G == 31))
                          for oc4 in range(4):
                              oc = 4 * half + oc4
                              nc.vector.scalar_tensor_tensor(
                                  X[:, oc], xps[:, oc4], b2_t[:, oc:oc + 1],
                                  ao_t[:, oc],
                                  op0=ALU.add, op1=ALU.add)
                      for of in range(4):
                          ofps = p2ps.tile([128, SUB], F32, tag="hps")
                          for c in range(8):
                              nc.tensor.matmul(
                                  ofps[:], wf_sb[:, c, 128 * of:128 * of + 128],
                                  X[:, c], start=(c == 0), stop=(c == 7))
                          outt = p2b.tile([128, SUB], FP16, tag="outt")
                          nc.vector.tensor_scalar_add(outt[:], ofps[:],
                                                      bf_t[:, of:of + 1])
                          nc.sync.dma_start(
                              outT[128 * of:128 * (of + 1),
                                   rowoff:rowoff + SUB], outt[:])
    nc.compile()
    return nc


def _prep_host(inputs):
    """Fold LN gains + attention scale into weights; build per-core maps."""
    f64 = np.float64
    gq, bq_ln = inputs["ln_q_g"].astype(f64), inputs["ln_q_b"].astype(f64)
    gkv = inputs["ln_kv_g"].astype(f64)
    bkv_ln = inputs["ln_kv_b"].astype(f64)
    ga, ba_ln = inputs["ln_a_g"].astype(f64), inputs["ln_a_b"].astype(f64)
    Wq, Wk, Wv = (np.asarray(inputs[k], f64) for k in ("Wq", "Wk", "Wv"))
    Wo, W1, W2, Wf = (np.asarray(inputs[k], f64) for k in ("Wo", "W1", "W2", "Wf"))
    bq_, bv_ = (np.asarray(inputs[k], f64) for k in ("bq", "bv"))
    bo_, b1_, b2_, bf_ = (np.asarray(inputs[k], f64)
                          for k in ("bo", "b1", "b2", "bf"))

    sc = 1.0 / np.sqrt(DH)
    Wq_e = (gq[:, None] * Wq) * sc
    bq_e = (bq_ln @ Wq + bq_) * sc
    Wk_e = gkv[:, None] * Wk
    Wv_e = gkv[:, None] * Wv
    bv_e = bkv_ln @ Wv + bv_
    W1_e = ga[:, None] * W1
    b1_e = ba_ln @ W1 + b1_

    qbias = bool(np.abs(bq_e).max() > 1e-12)

    perm = _query_perm()
    f32 = np.float32
    query = np.asarray(inputs["query"], f32)
    z = np.asarray(inputs["z"], f32)
    maps = []
    shared = {
        "wo": np.ascontiguousarray(Wo.astype(np.float16)),
        "w1": np.ascontiguousarray(W1_e.astype(np.float16)),
        "w2": np.ascontiguousarray(W2.astype(np.float16)),
        "wf": np.ascontiguousarray(Wf.astype(np.float16)),
        "bo": np.ascontiguousarray(bo_.reshape(8, 128).T.astype(f32)),
        "b1": np.ascontiguousarray(b1_e.reshape(32, 128).T.astype(f32)),
        "b2": np.ascontiguousarray(b2_.reshape(8, 128).T.astype(f32)),
        "bfp": np.ascontiguousarray(bf_.reshape(4, 128).T.astype(f32)),
    }
    for core in range(N_CORES):
        b, g = divmod(core, 2)
        hs = slice(512 * g, 512 * (g + 1))
        m = dict(shared)
        nwq_v = -(Wq_e[:, hs].sum(axis=0))
        m.update({
            "qT": np.ascontiguousarray(query[b][perm].T.astype(np.float16)),
            "zT": np.ascontiguousarray(z[b].T.astype(np.float16)),
            "wq": np.ascontiguousarray(Wq_e[:, hs].astype(np.float16)),
            "wk": np.ascontiguousarray(Wk_e[:, hs].astype(np.float16)),
            "wv": np.ascontiguousarray(Wv_e[:, hs].astype(np.float16)),
            "nwq": np.ascontiguousarray(nwq_v.reshape(HPC, 128).T.astype(f32)),
            "bvb": np.broadcast_to(bv_e[hs].astype(f32), (128, 512)).copy(),
        })
        if qbias:
            m["bqh"] = np.ascontiguousarray(
                bq_e[hs].reshape(HPC, 128).T.astype(f32))
        maps.append(m)
    return maps, qbias


def kernel(**inputs):
    assert bool(np.all(inputs["query_mask"])), \
        "kernel specialization assumes all-ones query_mask"
    maps, qbias = _prep_host(inputs)
    key = ("nc", qbias)
    if key not in _CACHE:
        _CACHE[key] = build(qbias=qbias)
    nc = _CACHE[key]
    res = bass_utils.run_bass_kernel_spmd(nc, maps, core_ids=list(range(N_CORES)))
    out = np.empty((B, Q, OUT_C), dtype=np.float32)
    for core in range(N_CORES):
        b, g = divmod(core, 2)
        out[b, ROWS * g:ROWS * (g + 1), :] = res.results[core]["outT"].T
    return out
